# revision 22
# baseline (speedup 1.0000x reference)
"""DSNT + JSD + distance double loss on 8 TRN2 NeuronCores.

Data-parallel: batch 64 is split into 8 shards of 8 batches; each core
computes per-partition/per-slice partial reductions over its 16 (b,c)
heatmap slices; the host finishes the tiny combine in float64.

Per (b,c) slice (512x512 -> SBUF [128, 2048], partition p holds rows
h in {4p..4p+3}):
  e    = exp(x) in fp16       ACT, accum -> per-partition rowsums
  cols = [ones|ys]^T @ e      PE fp16 matmul -> PSUM [2,512] col sums,
                              ACT-copied into an SBUF staging row; the
                              host does the xs-dot for px/py in f64
  S    = ones^T rowe; 1/S broadcast via PE matmul (the Pool engine must
                              run ONLY TensorTensor: every distinct Pool
                              op type swaps Q7 ucode, ~6us per swap)
  p    = e*invS               DVE tensor_scalar (2-byte 4x mode)
  m2   = p + t                Pool tensor_tensor (the only Pool op)
  l    = ln(m2)               ACT bf16
  w    = m2*l                 DVE bf16 2x
  sum(m2), sum(w) over cols   PE bf16 matmuls into whole-run PSUM rows
  sum(m2^2)                   ACT Square with accum
  argmax(t): DVE max8 + max_index (exact first-occurrence); the [P,8]
  results are DMA'd straight to the host, which resolves the global
  argmax and all remaining scalar math (ed, distances, jsd) in f64.

The loop is software-pipelined two slices deep (front / mid / back) so
no engine stalls on the exp->S->inv->p->m2 cross-engine chain.
"""

import math
import os

import numpy as np

import concourse.bacc as bacc
import concourse.bass as bass
import concourse.mybir as mybir
import concourse.tile as tile
from concourse.bass_utils import run_bass_kernel_spmd

F32 = mybir.dt.float32
BF16 = mybir.dt.bfloat16
FP16 = mybir.dt.float16
U32 = mybir.dt.uint32
ALU = mybir.AluOpType
ACTF = mybir.ActivationFunctionType
AX = mybir.AxisListType

B, C, H, W = 64, 2, 512, 512
N_CORES = 8
B_SH = B // N_CORES          # 8 batches per core
NSL = B_SH * C               # 16 slices per core
P = 128                      # SBUF partitions
FD = (H * W) // P            # 2048 free elements per partition
SUB = W                      # 512-wide sub-columns (4 per row)
NSUB = FD // SUB             # 4

_CACHE = {}
LAST_RESULTS = None


def _constants():
    # ys is exact in fp16: (h+1)/512 = (h+1) * 2^-9 with h+1 <= 512 < 2^11
    hidx = (np.arange(P, dtype=np.float32)[:, None] * NSUB
            + np.arange(NSUB, dtype=np.float32)[None, :])
    ys = (hidx + 1.0) / H                                   # [128, 4]
    oy = np.zeros((P, 2 * NSUB), dtype=np.float16)
    for j in range(NSUB):
        oy[:, 2 * j] = 1.0
        oy[:, 2 * j + 1] = ys[:, j].astype(np.float16)
    return {"oy_c": oy}


def _patch_act_tables():
    """Steer the act-table chooser so Exp/Ln/Square/Copy all live in the
    single `natural_log_exp_and_others` set — otherwise the per-slice
    Exp->Ln alternation reloads tables (~1.3us each, 32x per core).
    Set ids stay aligned with act_info.json (same list, same order; only
    membership of the non-preferred sets is pruned)."""
    if _CACHE.get("act_patched"):
        return
    import concourse.hw_specs as hw_specs

    orig = hw_specs.get_activation_tables
    hot = {ACTF.Exp, ACTF.Ln, ACTF.Square, ACTF.Copy, ACTF.Identity}

    def patched(module_arch):
        tabs = orig(module_arch)
        out = {}
        for name, funcs in tabs.items():
            if name == "natural_log_exp_and_others":
                out[name] = set(funcs)
            else:
                out[name] = set(funcs) - hot
        return out

    hw_specs.get_activation_tables = patched
    bacc.get_activation_tables = patched
    _CACHE["act_patched"] = True


def build_program():
    """Build (once) the single-core Bass/Tile program run SPMD on 8 cores."""
    if "nc" in _CACHE:
        return _CACHE["nc"]

    _patch_act_tables()
    nc = bacc.Bacc("TRN2", target_bir_lowering=False, debug=False,
                   num_devices=N_CORES)

    x_d = nc.dram_tensor("x", [NSL, P, FD], F32, kind="ExternalInput").ap()
    t_d = nc.dram_tensor("t", [NSL, P, FD], F32, kind="ExternalInput").ap()
    oy_d = nc.dram_tensor("oy_c", [P, 2 * NSUB], FP16,
                          kind="ExternalInput").ap()
    big_d = nc.dram_tensor("out_big", [P, NSL], F32,
                           kind="ExternalOutput").ap()
    max_d = nc.dram_tensor("out_max", [NSL, P, 8], F32,
                           kind="ExternalOutput").ap()
    idx_d = nc.dram_tensor("out_idx", [NSL, P, 8], U32,
                           kind="ExternalOutput").ap()
    cols_d = nc.dram_tensor("out_cols", [2, NSL * W], F32,
                            kind="ExternalOutput").ap()
    sml_d = nc.dram_tensor("out_small", [1, NSL + 2 * W], F32,
                           kind="ExternalOutput").ap()

    with tile.TileContext(nc) as tc:
        _emit(nc, tc, x_d, t_d, oy_d, big_d, max_d, idx_d, cols_d, sml_d)

    nc.compile()
    _CACHE["nc"] = nc
    return nc


def _emit(nc, tc, x_d, t_d, oy_d, big_d, max_d, idx_d, cols_d, sml_d):
    from contextlib import ExitStack
    ctx = ExitStack()
    with ctx:
        singles = ctx.enter_context(tc.tile_pool(name="singles", bufs=1))
        xp = ctx.enter_context(tc.tile_pool(name="xp", bufs=3))
        tp = ctx.enter_context(tc.tile_pool(name="tp", bufs=4))
        ep = ctx.enter_context(tc.tile_pool(name="ep", bufs=3))
        pp = ctx.enter_context(tc.tile_pool(name="pp", bufs=3))
        m2p = ctx.enter_context(tc.tile_pool(name="m2p", bufs=3))
        lp = ctx.enter_context(tc.tile_pool(name="lp", bufs=2))
        scr = ctx.enter_context(tc.tile_pool(name="scr", bufs=2))
        sm = ctx.enter_context(tc.tile_pool(name="sm", bufs=4))
        pcols = ctx.enter_context(
            tc.tile_pool(name="pcols", bufs=2, space="PSUM"))
        pwsum = ctx.enter_context(
            tc.tile_pool(name="pwsum", bufs=1, space="PSUM"))
        pm2sum = ctx.enter_context(
            tc.tile_pool(name="pm2sum", bufs=1, space="PSUM"))
        ps_s = ctx.enter_context(
            tc.tile_pool(name="ps_s", bufs=2, space="PSUM"))
        ps_inv = ctx.enter_context(
            tc.tile_pool(name="ps_inv", bufs=2, space="PSUM"))

        # ---- constants: 1 DMA on the ACT queue + 3 memsets ----
        oy_sb = singles.tile([P, 2 * NSUB], FP16)
        nc.scalar.dma_start(out=oy_sb, in_=oy_d)
        ones_sb = singles.tile([P, 1], F32)
        nc.vector.memset(ones_sb, 1.0)
        onesb_sb = singles.tile([P, 1], BF16)
        nc.vector.memset(onesb_sb, 1.0)
        onesr_sb = singles.tile([1, P], F32)
        nc.vector.memset(onesr_sb, 1.0)

        # ---- result accumulators (DMA'd out at the end) ----
        big = singles.tile([P, NSL], F32)        # sum(m2^2) per partition
        # small row: [invS(16) | wrow(512) | m2row(512)]
        small = singles.tile([1, NSL + 2 * W], F32)
        # per-slice [ones|ys]^T e col sums staged for one DMA at the end
        cols_sb = singles.tile([2, NSL * W], F32)
        wsum_ps = pwsum.tile([1, W], F32)        # col sums of w = m2*l
        m2sum_ps = pm2sum.tile([1, W], F32)      # col sums of m2

        state = [None] * NSL

        def front(s):
            st = {}
            x_sb = xp.tile([P, FD], F32, tag="x")
            nc.sync.dma_start(out=x_sb, in_=x_d[s])
            t_sb = tp.tile([P, FD], F32, tag="t")
            nc.sync.dma_start(out=t_sb, in_=t_d[s])

            # argmax of target (exact, first occurrence per partition)
            mx8 = sm.tile([P, 8], F32, tag="mx8")
            nc.vector.max(out=mx8, in_=t_sb)

            # softmax stats
            e_sb = ep.tile([P, FD], FP16, tag="e")
            rowe = sm.tile([P, 1], F32, tag="rowe")
            nc.scalar.activation(out=e_sb, in_=x_sb, func=ACTF.Exp,
                                 accum_out=rowe)
            s_ps = ps_s.tile([1, 1], F32, tag="s_ps")
            nc.tensor.matmul(s_ps[0:1, 0:1], lhsT=ones_sb[:, 0:1],
                             rhs=rowe[:, 0:1], start=True, stop=True)
            # 1/S into the small out row (also the host's copy)
            nc.vector.reciprocal(out=small[0:1, s:s + 1],
                                 in_=s_ps[0:1, 0:1])
            # broadcast 1/S to all partitions through the PE
            invs_ps = ps_inv.tile([P, 1], F32, tag="invs_ps")
            nc.tensor.matmul(invs_ps[:, 0:1], lhsT=onesr_sb[0:1, :],
                             rhs=small[0:1, s:s + 1],
                             start=True, stop=True)

            ix8 = sm.tile([P, 8], U32, tag="ix8")
            nc.vector.max_index(out=ix8, in_max=mx8, in_values=t_sb)
            invs_sb = sm.tile([P, 1], F32, tag="invs_sb")
            nc.vector.tensor_copy(out=invs_sb, in_=invs_ps[:, 0:1])

            # p = e * invS on DVE (same-dtype 2-byte tensor_scalar: 4x)
            p_sb = pp.tile([P, FD], FP16, tag="p")
            nc.vector.tensor_scalar(out=p_sb, in0=e_sb,
                                    scalar1=invs_sb[:, 0:1], scalar2=None,
                                    op0=ALU.mult)

            # fused col sums: row 0 = sum_h e, row 1 = sum_h ys[h]*e
            cols_ps = pcols.tile([2, W], F32, tag="cols")
            for j in range(NSUB):
                nc.tensor.matmul(
                    cols_ps[0:2, :],
                    lhsT=oy_sb[:, 2 * j:2 * j + 2],
                    rhs=e_sb[:, j * SUB:(j + 1) * SUB],
                    start=(j == 0), stop=(j == NSUB - 1))

            # per-partition argmax results straight out to the host
            nc.sync.dma_start(out=max_d[s], in_=mx8)
            nc.sync.dma_start(out=idx_d[s], in_=ix8)
            st["p"] = p_sb
            st["t"] = t_sb
            st["cols"] = cols_ps
            return st

        def mid(s):
            st = state[s]
            # m2 = p + t on Pool
            m2_sb = m2p.tile([P, FD], BF16, tag="m2")
            nc.gpsimd.tensor_tensor(out=m2_sb, in0=st["p"], in1=st["t"],
                                    op=ALU.add)
            # PSUM cols -> SBUF staging (ACT has the spare PSUM port)
            nc.scalar.activation(out=cols_sb[0:2, s * W:(s + 1) * W],
                                 in_=st["cols"][0:2, :], func=ACTF.Copy)
            st["m2"] = m2_sb

        def back(s):
            st = state[s]
            m2_sb = st["m2"]
            l_sb = lp.tile([P, FD], BF16, tag="l")
            nc.scalar.activation(out=l_sb, in_=m2_sb, func=ACTF.Ln)
            w_sb = scr.tile([P, FD], BF16, tag="w")
            nc.vector.tensor_tensor(out=w_sb, in0=m2_sb, in1=l_sb,
                                    op=ALU.mult)
            # all wsum matmuls first: they gate on the DVE w-mult being
            # done, so the m2sum reads of m2_sb can't race the DVE pass
            # that is still reading m2_sb (shared SBUF port contention)
            for j in range(NSUB):
                nc.tensor.matmul(
                    wsum_ps[0:1, :], lhsT=onesb_sb[:, 0:1],
                    rhs=w_sb[:, j * SUB:(j + 1) * SUB],
                    start=(s == 0 and j == 0),
                    stop=(s == NSL - 1 and j == NSUB - 1),
                    skip_group_check=True)
            for j in range(NSUB):
                nc.tensor.matmul(
                    m2sum_ps[0:1, :], lhsT=onesb_sb[:, 0:1],
                    rhs=m2_sb[:, j * SUB:(j + 1) * SUB],
                    start=(s == 0 and j == 0),
                    stop=(s == NSL - 1 and j == NSUB - 1),
                    skip_group_check=True)
            sq_sb = scr.tile([P, FD], BF16, tag="sq")
            nc.scalar.activation(
                out=sq_sb, in_=m2_sb, func=ACTF.Square,
                accum_out=big[:, s:s + 1])

        for s in range(NSL):
            state[s] = front(s)
            if s >= 1:
                mid(s - 1)
            if s >= 2:
                back(s - 2)
        mid(NSL - 1)
        back(NSL - 2)
        back(NSL - 1)

        # evacuate the whole-run PSUM accumulator rows
        nc.vector.tensor_copy(out=small[0:1, NSL:NSL + W],
                              in_=wsum_ps[0:1, :])
        nc.vector.tensor_copy(out=small[0:1, NSL + W:NSL + 2 * W],
                              in_=m2sum_ps[0:1, :])

        nc.sync.dma_start(out=big_d, in_=big)
        nc.sync.dma_start(out=cols_d, in_=cols_sb)
        nc.sync.dma_start(out=sml_d, in_=small)


def make_in_maps(input, target):
    consts = _constants()
    in_maps = []
    for i in range(N_CORES):
        xs = np.ascontiguousarray(
            input[i * B_SH:(i + 1) * B_SH].reshape(NSL, P, FD))
        ts = np.ascontiguousarray(
            target[i * B_SH:(i + 1) * B_SH].reshape(NSL, P, FD))
        m = {"x": xs, "t": ts}
        m.update(consts)
        in_maps.append(m)
    return in_maps


def _host_combine(res):
    """Finish the loss from per-core partial reductions, in float64."""
    xs = (np.arange(W, dtype=np.float64) + 1.0) / W
    s_total = 0.0
    for i in range(N_CORES):
        r = res.results[i]
        big = np.asarray(r["out_big"], dtype=np.float64)     # [P, 16]
        pmax = np.asarray(r["out_max"],
                          dtype=np.float64)[:, :, 0].T      # [P, 16]
        idx = np.asarray(r["out_idx"],
                         dtype=np.uint32)[:, :, 0].T        # [P, 16]
        cols = np.asarray(r["out_cols"], dtype=np.float64)   # [2, 16*512]
        sml = np.asarray(r["out_small"], dtype=np.float64)   # [1, 1040]

        sq_tot = big.sum()
        w_tot = sml[0, NSL:NSL + W].sum()
        m_tot = sml[0, NSL + W:NSL + 2 * W].sum()

        jsd_tot = (0.5 * w_tot - 0.5 * math.log(2.0) * m_tot
                   - 0.25 * sq_tot) / float(H * W)

        invs = sml[0, 0:NSL]                                 # [16]
        px = (cols[0].reshape(NSL, W) @ xs) * invs
        py = cols[1].reshape(NSL, W).sum(axis=1) * invs

        # global argmax per slice: max over partitions, first occurrence
        # (min flat index among tied partitions; flat = p*FD + idx)
        flat = (np.arange(P, dtype=np.int64)[:, None] * FD
                + idx.astype(np.int64))                      # [P, 16]
        col_max = pmax.max(axis=0)                           # [16]
        tied = pmax == col_max[None, :]
        flat_sel = np.where(tied, flat, np.int64(1) << 40).min(axis=0)
        tx = ((flat_sel % W) + 1.0) / W
        ty = ((flat_sel // W) + 1.0) / H

        ed = np.sqrt((tx - px) ** 2 + (ty - py) ** 2).sum()

        px0, px1 = px[0::2], px[1::2]
        py0, py1 = py[0::2], py[1::2]
        tx0, tx1 = tx[0::2], tx[1::2]
        ty0, ty1 = ty[0::2], ty[1::2]
        pred_d = np.sqrt((px0 - px1) ** 2 + (py0 - py1) ** 2)
        true_d = np.sqrt((tx0 - tx1) ** 2 + (ty0 - ty1) ** 2)
        dd = np.abs(pred_d - true_d).sum()

        s_total += ed + jsd_tot + dd
    return np.array([s_total / B], dtype=np.float32)


def kernel(input, target):
    global LAST_RESULTS
    input = np.asarray(input, dtype=np.float32)
    target = np.asarray(target, dtype=np.float32)
    nc = build_program()
    in_maps = make_in_maps(input, target)
    res = run_bass_kernel_spmd(nc, in_maps, list(range(N_CORES)))
    LAST_RESULTS = res
    return _host_combine(res)


# revision 32
# speedup vs baseline: 1.2077x; 1.2077x over previous
"""DSNT + JSD + distance double loss on 8 TRN2 NeuronCores.

Data-parallel: batch 64 is split into 8 shards of 8 batches; each core
computes per-partition/per-slice partial reductions over its 16 (b,c)
heatmap slices; the host finishes the tiny combine in float64.

Per (b,c) slice (512x512 -> SBUF [128, 2048], partition p holds rows
h in {4p..4p+3}):
  e    = exp(x) in fp16       ACT, accum -> per-partition rowsums
  cols = [ones|ys]^T @ e      PE fp16 matmul -> PSUM [2,512] col sums,
                              ACT-copied to an SBUF staging row; the
                              host does the xs-dot for px/py AND takes
                              S = sum(cols row0) in f64
  S    = ones^T @ rowe -> [1,1]; DVE reciprocal; onesr^T @ invS
                              broadcasts 1/S to [P,1] (1-col weights
                              keep the PE LDWEIGHTS cost trivial)
  m2   = e*invS + t           DVE fused scalar_tensor_tensor with
                              accum_out -> sum(m2) per partition
  l    = ln(m2)               ACT bf16
  w    = m2*l                 DVE bf16 2x; PE col-sums -> whole-run PSUM
  sum(m2^2)                   ACT Square with accum
  argmax(t): DVE max8 + exact max_index (first occurrence); [P,8]
  results DMA straight to the host, which resolves the global argmax
  and the remaining scalar math (ed, pair distances, jsd) in f64.

The Pool engine is deliberately UNUSED: its TensorTensor takes 4.1us
per big op (0.42 sw efficiency) and its SBUF port is the DVE's second
port, so any overlap halves DVE 2-port throughput (measured: the
m2*l mult went 1.2us -> 2.4us whenever Pool ran). TT(max) and stt are
also rejected by walrus on Pool, which rules out the cheap offloads.

The loop is software-pipelined: every consumed value is at least one
slice-period old, so no engine stalls on cross-engine chains.
"""

import math
import os

import numpy as np

import concourse.bacc as bacc
import concourse.bass as bass
import concourse.mybir as mybir
import concourse.tile as tile
from concourse.bass_utils import run_bass_kernel_spmd

F32 = mybir.dt.float32
BF16 = mybir.dt.bfloat16
FP16 = mybir.dt.float16
U32 = mybir.dt.uint32
ALU = mybir.AluOpType
ACTF = mybir.ActivationFunctionType
AX = mybir.AxisListType

B, C, H, W = 64, 2, 512, 512
N_CORES = 8
B_SH = B // N_CORES          # 8 batches per core
NSL = B_SH * C               # 16 slices per core
P = 128                      # SBUF partitions
FD = (H * W) // P            # 2048 free elements per partition
SUB = W                      # 512-wide sub-columns (4 per row)
NSUB = FD // SUB             # 4
HF = FD // 2                 # 1024
QF = FD // 4                 # 512

_CACHE = {}
LAST_RESULTS = None


def _constants():
    # ys is exact in fp16: (h+1)/512 = (h+1) * 2^-9 with h+1 <= 512 < 2^11
    hidx = (np.arange(P, dtype=np.float32)[:, None] * NSUB
            + np.arange(NSUB, dtype=np.float32)[None, :])
    ys = (hidx + 1.0) / H                                   # [128, 4]
    oy = np.zeros((P, 2 * NSUB), dtype=np.float16)
    for j in range(NSUB):
        oy[:, 2 * j] = 1.0
        oy[:, 2 * j + 1] = ys[:, j].astype(np.float16)
    return {"oy_c": oy}


def _patch_act_tables():
    """Steer the act-table chooser so Exp/Ln/Square/Copy all live in the
    single `natural_log_exp_and_others` set — otherwise the per-slice
    Exp->Ln alternation reloads tables (~1.3us each, 32x per core).
    Set ids stay aligned with act_info.json (same list, same order; only
    membership of the non-preferred sets is pruned)."""
    if _CACHE.get("act_patched"):
        return
    import concourse.hw_specs as hw_specs

    orig = hw_specs.get_activation_tables
    hot = {ACTF.Exp, ACTF.Ln, ACTF.Square, ACTF.Copy, ACTF.Identity}

    def patched(module_arch):
        tabs = orig(module_arch)
        out = {}
        for name, funcs in tabs.items():
            if name == "natural_log_exp_and_others":
                out[name] = set(funcs)
            else:
                out[name] = set(funcs) - hot
        return out

    hw_specs.get_activation_tables = patched
    bacc.get_activation_tables = patched
    _CACHE["act_patched"] = True


def build_program():
    """Build (once) the single-core Bass/Tile program run SPMD on 8 cores."""
    if "nc" in _CACHE:
        return _CACHE["nc"]

    _patch_act_tables()
    nc = bacc.Bacc("TRN2", target_bir_lowering=False, debug=False,
                   num_devices=N_CORES)

    x_d = nc.dram_tensor("x", [NSL, P, FD], F32, kind="ExternalInput").ap()
    t_d = nc.dram_tensor("t", [NSL, P, FD], F32, kind="ExternalInput").ap()
    oy_d = nc.dram_tensor("oy_c", [P, 2 * NSUB], FP16,
                          kind="ExternalInput").ap()
    big_d = nc.dram_tensor("out_big", [P, 2 * NSL], F32,
                           kind="ExternalOutput").ap()
    max_d = nc.dram_tensor("out_max", [NSL, P, 8], F32,
                           kind="ExternalOutput").ap()
    idx_d = nc.dram_tensor("out_idx", [NSL, P, 8], U32,
                           kind="ExternalOutput").ap()
    cols_d = nc.dram_tensor("out_cols", [2, NSL * W], F32,
                            kind="ExternalOutput").ap()
    sml_d = nc.dram_tensor("out_small", [1, W], F32,
                           kind="ExternalOutput").ap()

    with tile.TileContext(nc) as tc:
        _emit(nc, tc, x_d, t_d, oy_d, big_d, max_d, idx_d, cols_d, sml_d)

    nc.compile()
    _CACHE["nc"] = nc
    return nc


def _emit(nc, tc, x_d, t_d, oy_d, big_d, max_d, idx_d, cols_d, sml_d):
    from contextlib import ExitStack
    ctx = ExitStack()
    with ctx:
        singles = ctx.enter_context(tc.tile_pool(name="singles", bufs=1))
        xp = ctx.enter_context(tc.tile_pool(name="xp", bufs=3))
        tp = ctx.enter_context(tc.tile_pool(name="tp", bufs=4))
        ep = ctx.enter_context(tc.tile_pool(name="ep", bufs=3))
        m2p = ctx.enter_context(tc.tile_pool(name="m2p", bufs=3))
        lp = ctx.enter_context(tc.tile_pool(name="lp", bufs=3))
        scr = ctx.enter_context(tc.tile_pool(name="scr", bufs=2))
        sm = ctx.enter_context(tc.tile_pool(name="sm", bufs=4))
        pcols = ctx.enter_context(
            tc.tile_pool(name="pcols", bufs=2, space="PSUM"))
        pwsum = ctx.enter_context(
            tc.tile_pool(name="pwsum", bufs=1, space="PSUM"))
        ps_s = ctx.enter_context(
            tc.tile_pool(name="ps_s", bufs=2, space="PSUM"))
        ps_inv = ctx.enter_context(
            tc.tile_pool(name="ps_inv", bufs=2, space="PSUM"))

        # ---- constants: 1 DMA on the ACT queue + 3 memsets ----
        oy_sb = singles.tile([P, 2 * NSUB], FP16)
        nc.scalar.dma_start(out=oy_sb, in_=oy_d)
        onesb_sb = singles.tile([P, 1], BF16)
        nc.vector.memset(onesb_sb, 1.0)
        ones_sb = singles.tile([P, 1], F32)
        nc.vector.memset(ones_sb, 1.0)
        onesr_sb = singles.tile([1, P], F32)
        nc.vector.memset(onesr_sb, 1.0)

        # ---- result accumulators (DMA'd out at the end) ----
        # big: [:, 0:16] sum(m2^2), [:, 16:32] sum(m2)
        big = singles.tile([P, 2 * NSL], F32)
        # per-slice [ones|ys]^T e col sums staged for one DMA at the end
        cols_sb = singles.tile([2, NSL * W], F32)
        small = singles.tile([1, W], F32)         # w col sums
        wsum_ps = pwsum.tile([1, W], F32)         # col sums of w = m2*l

        state = [None] * NSL

        def prefetch(s):
            st = {}
            x_sb = xp.tile([P, FD], F32, tag="x")
            nc.sync.dma_start(out=x_sb, in_=x_d[s])
            t_sb = tp.tile([P, FD], F32, tag="t")
            nc.sync.dma_start(out=t_sb, in_=t_d[s])
            st["x"] = x_sb
            st["t"] = t_sb
            return st

        def front(s):
            st = state[s]
            t_sb = st["t"]
            mx8 = sm.tile([P, 8], F32, tag="mx8")
            nc.vector.max(out=mx8, in_=t_sb)
            ix8 = sm.tile([P, 8], U32, tag="ix8")
            nc.vector.max_index(out=ix8, in_max=mx8, in_values=t_sb)

            e_sb = ep.tile([P, FD], FP16, tag="e")
            rowe = sm.tile([P, 1], F32, tag="rowe")
            nc.scalar.activation(out=e_sb, in_=st["x"], func=ACTF.Exp,
                                 accum_out=rowe)
            # S -> 1/S -> broadcast [P,1] (all weight loads are 1-col)
            s_ps = ps_s.tile([1, 1], F32, tag="s_ps")
            nc.tensor.matmul(s_ps[0:1, 0:1], lhsT=ones_sb[:, 0:1],
                             rhs=rowe[:, 0:1], start=True, stop=True)
            inv1 = sm.tile([1, 1], F32, tag="inv1")
            nc.vector.reciprocal(out=inv1, in_=s_ps[0:1, 0:1])
            invs_ps = ps_inv.tile([P, 1], F32, tag="invs_ps")
            nc.tensor.matmul(invs_ps[:, 0:1], lhsT=onesr_sb[0:1, :],
                             rhs=inv1[0:1, 0:1], start=True, stop=True)
            invs_sb = sm.tile([P, 1], F32, tag="invs_sb")
            nc.vector.tensor_copy(out=invs_sb, in_=invs_ps[:, 0:1])

            # fused col sums: row 0 = sum_h e, row 1 = sum_h ys[h]*e
            cols_ps = pcols.tile([2, W], F32, tag="cols")
            for j in range(NSUB):
                nc.tensor.matmul(
                    cols_ps[0:2, :],
                    lhsT=oy_sb[:, 2 * j:2 * j + 2],
                    rhs=e_sb[:, j * SUB:(j + 1) * SUB],
                    start=(j == 0), stop=(j == NSUB - 1))

            # per-partition argmax results straight out to the host
            nc.sync.dma_start(out=max_d[s], in_=mx8)
            nc.sync.dma_start(out=idx_d[s], in_=ix8)
            st["e"] = e_sb
            st["invs"] = invs_sb
            st["cols"] = cols_ps

        def mid_stt(s):
            st = state[s]
            # m2 = e*invS + t in one DVE op; accum -> sum(m2)/partition
            m2_sb = m2p.tile([P, FD], BF16, tag="m2")
            nc.vector.scalar_tensor_tensor(
                out=m2_sb, in0=st["e"], scalar=st["invs"][:, 0:1],
                in1=st["t"], op0=ALU.mult, op1=ALU.add,
                accum_out=big[:, NSL + s:NSL + s + 1])
            st["m2"] = m2_sb

        def mid_cols(s):
            st = state[s]
            # PSUM cols -> SBUF staging (ACT has the spare PSUM port)
            nc.scalar.activation(out=cols_sb[0:2, s * W:(s + 1) * W],
                                 in_=st["cols"][0:2, :], func=ACTF.Copy)

        def back(s):
            st = state[s]
            m2_sb = st["m2"]
            l_sb = lp.tile([P, FD], BF16, tag="l")
            nc.scalar.activation(out=l_sb, in_=m2_sb, func=ACTF.Ln)
            w_sb = scr.tile([P, FD], BF16, tag="w")
            nc.vector.tensor_tensor(out=w_sb, in0=m2_sb, in1=l_sb,
                                    op=ALU.mult)
            for j in range(NSUB):
                nc.tensor.matmul(
                    wsum_ps[0:1, :], lhsT=onesb_sb[:, 0:1],
                    rhs=w_sb[:, j * SUB:(j + 1) * SUB],
                    start=(s == 0 and j == 0),
                    stop=(s == NSL - 1 and j == NSUB - 1),
                    skip_group_check=True)
            sq_sb = scr.tile([P, FD], BF16, tag="sq")
            nc.scalar.activation(
                out=sq_sb, in_=m2_sb, func=ACTF.Square,
                accum_out=big[:, s:s + 1])

        state[0] = prefetch(0)
        for s in range(NSL):
            if s + 1 < NSL:
                state[s + 1] = prefetch(s + 1)
            if s >= 1:
                mid_stt(s - 1)
            front(s)
            if s >= 2:
                back(s - 2)
            if s >= 1:
                mid_cols(s - 1)
        mid_stt(NSL - 1)
        mid_cols(NSL - 1)
        back(NSL - 2)
        back(NSL - 1)

        # evacuate the whole-run w col sums
        nc.vector.tensor_copy(out=small[0:1, 0:W], in_=wsum_ps[0:1, :])

        nc.sync.dma_start(out=big_d, in_=big)
        nc.sync.dma_start(out=cols_d, in_=cols_sb)
        nc.sync.dma_start(out=sml_d, in_=small)


def make_in_maps(input, target):
    consts = _constants()
    in_maps = []
    for i in range(N_CORES):
        xs = np.ascontiguousarray(
            input[i * B_SH:(i + 1) * B_SH].reshape(NSL, P, FD))
        ts = np.ascontiguousarray(
            target[i * B_SH:(i + 1) * B_SH].reshape(NSL, P, FD))
        m = {"x": xs, "t": ts}
        m.update(consts)
        in_maps.append(m)
    return in_maps


def _host_combine(res):
    """Finish the loss from per-core partial reductions, in float64."""
    xs = (np.arange(W, dtype=np.float64) + 1.0) / W
    s_total = 0.0
    for i in range(N_CORES):
        r = res.results[i]
        big = np.asarray(r["out_big"], dtype=np.float64)     # [P, 32]
        pmax = np.asarray(r["out_max"],
                          dtype=np.float64)[:, :, 0].T      # [P, 16]
        idx = np.asarray(r["out_idx"],
                         dtype=np.uint32)[:, :, 0].T        # [P, 16]
        cols = np.asarray(r["out_cols"], dtype=np.float64)   # [2, 16*512]
        sml = np.asarray(r["out_small"], dtype=np.float64)   # [1, 512]

        sq_tot = big[:, 0:NSL].sum()
        m_tot = big[:, NSL:2 * NSL].sum()
        w_tot = sml[0, 0:W].sum()

        jsd_tot = (0.5 * w_tot - 0.5 * math.log(2.0) * m_tot
                   - 0.25 * sq_tot) / float(H * W)

        c0 = cols[0].reshape(NSL, W)
        c1 = cols[1].reshape(NSL, W)
        S = c0.sum(axis=1)                                   # [16]
        px = (c0 @ xs) / S
        py = c1.sum(axis=1) / S

        # global argmax per slice: max over partitions, first occurrence
        # (min flat index among tied partitions; flat = p*FD + idx)
        flat = (np.arange(P, dtype=np.int64)[:, None] * FD
                + idx.astype(np.int64))                      # [P, 16]
        col_max = pmax.max(axis=0)                           # [16]
        tied = pmax == col_max[None, :]
        flat_sel = np.where(tied, flat, np.int64(1) << 40).min(axis=0)
        tx = ((flat_sel % W) + 1.0) / W
        ty = ((flat_sel // W) + 1.0) / H

        ed = np.sqrt((tx - px) ** 2 + (ty - py) ** 2).sum()

        px0, px1 = px[0::2], px[1::2]
        py0, py1 = py[0::2], py[1::2]
        tx0, tx1 = tx[0::2], tx[1::2]
        ty0, ty1 = ty[0::2], ty[1::2]
        pred_d = np.sqrt((px0 - px1) ** 2 + (py0 - py1) ** 2)
        true_d = np.sqrt((tx0 - tx1) ** 2 + (ty0 - ty1) ** 2)
        dd = np.abs(pred_d - true_d).sum()

        s_total += ed + jsd_tot + dd
    return np.array([s_total / B], dtype=np.float32)


def kernel(input, target):
    global LAST_RESULTS
    input = np.asarray(input, dtype=np.float32)
    target = np.asarray(target, dtype=np.float32)
    nc = build_program()
    in_maps = make_in_maps(input, target)
    res = run_bass_kernel_spmd(nc, in_maps, list(range(N_CORES)))
    LAST_RESULTS = res
    return _host_combine(res)
